# revision 40
# baseline (speedup 1.0000x reference)
"""CfC Liquid Cell kernel for Trainium2 (Bass/Tile), 8 NeuronCores.

Sharding: data-parallel over batch (B=8 -> 1 batch element per core).

Single fused loop over time chunks of T=256 (8 chunks), per core:
  - x is pre-transposed (and cast to bf16) on the host -> no PE transposes
  - in_proj matmuls (bf16 weights stationary, x^T moving) -> xz^T in PSUM
  - x_path half copied to SBUF (with causal halo), z half silu'd
  - depthwise causal conv = 4 shifted diagonal matmuls on PE; conv bias is
    folded into the silu ACT bias (no bias tap matmul)
  - head matmuls (bb/f1/f2/tau/decay/state_out) with 2-head block-diagonal
    64x64 weights -> full 128-partition tiles
  - sigmoid(u) = 0.5 + 0.5*tanh(u/2) via ACT scale=0.5; the scan carries
    H = 2*h (state_out weights pre-scaled by 0.5) so the gate algebra is
    only 5 fused DVE/GPSIMD passes:
       r2 = f2 - f1
       v  = (Tt + 1) * r2          # = 2*tau*(f2-f1)
       cand = 0.5*v + f1
       dd  = 0.5*Td + 0.5          # = decay
       cpn = (Td - 1) * cand       # = -2*(1-decay)*cand
       H_t = dd*H_{t-1} - cpn      # = dd*H + 2(1-dd)*cand
  - state_out bias + z-gating fused in one scalar_tensor_tensor that reads
    the state_out PSUM directly
  - out_proj with gated activations as the stationary operand -> y is
    produced time-major and DMA'd straight out
"""

import numpy as np

B, S, H = 8, 2048, 1024
NH, HD, NS, K = 16, 64, 64, 4
N_CORES = 8
T = 256             # time chunk
NB = S // T         # 8
P = 128

_CACHE = {}


def _build_program():
    import concourse.bacc as bacc
    import concourse.mybir as mybir
    import concourse.tile as tile

    F32 = mybir.dt.float32
    BF16 = mybir.dt.bfloat16
    AF = mybir.ActivationFunctionType
    ALU = mybir.AluOpType

    nc = bacc.Bacc("TRN2", target_bir_lowering=False, debug=False)

    xT_d = nc.dram_tensor("xT", (NB, P, 8, S // NB), BF16, kind="ExternalInput").ap()
    w_in_d = nc.dram_tensor("w_in", (P, 8, 2 * H), BF16, kind="ExternalInput").ap()
    cdiag_d = nc.dram_tensor("cdiag", (P, 8, K, P), BF16, kind="ExternalInput").ap()
    blk_d = nc.dram_tensor("blk", (P, 6, P), BF16, kind="ExternalInput").ap()
    w_out_d = nc.dram_tensor("w_out", (P, 8, H), BF16, kind="ExternalInput").ap()
    bias_d = nc.dram_tensor("bias", (P, 14), F32, kind="ExternalInput").ap()
    y_d = nc.dram_tensor("y", (S, H), F32, kind="ExternalOutput").ap()

    with tile.TileContext(nc) as tc:
        with tc.tile_pool(name="const", bufs=1) as cpool, \
             tc.tile_pool(name="pxT", bufs=2) as pxT0:
            # prefetch the first x chunk ahead of the big weight DMAs
            xT_first = pxT0.tile([P, 8, T], BF16, tag="xT", name="xT_first")
            nc.sync.dma_start(xT_first[:], xT_d[0])
            w_in_t = [cpool.tile([P, 2 * H], BF16, name=f"w_in{k}") for k in range(8)]
            for kt in range(8):
                eng = nc.scalar if kt % 2 == 0 else nc.sync
                eng.dma_start(w_in_t[kt][:], w_in_d[:, kt, :])
            bias = cpool.tile([P, 14], F32)
            nc.sync.dma_start(bias[:], bias_d[:])
            cdiag = cpool.tile([P, 8, K, P], BF16)
            nc.sync.dma_start(cdiag[:, 0:4], cdiag_d[:, 0:4])
            nc.scalar.dma_start(cdiag[:, 4:8], cdiag_d[:, 4:8])
            blk = cpool.tile([P, 6, P], BF16)
            nc.scalar.dma_start(blk[:], blk_d[:])
            w_out = cpool.tile([P, 8, H], BF16)
            nc.sync.dma_start(w_out[:, 0:4, :], w_out_d[:, 0:4, :])
            nc.scalar.dma_start(w_out[:, 4:8, :], w_out_d[:, 4:8, :])

            pxT = pxT0
            with \
                 tc.tile_pool(name="pxp", bufs=2) as pxp, \
                 tc.tile_pool(name="pzs", bufs=3) as pzs, \
                 tc.tile_pool(name="pxh", bufs=2) as pxh, \
                 tc.tile_pool(name="pbb", bufs=2) as pbb, \
                 tc.tile_pool(name="pgt", bufs=2) as pgt, \
                 tc.tile_pool(name="palg", bufs=1) as palg, \
                 tc.tile_pool(name="pH", bufs=2) as pH, \
                 tc.tile_pool(name="pgh", bufs=2) as pgh, \
                 tc.tile_pool(name="pysb", bufs=3) as pysb, \
                 tc.tile_pool(name="psA", bufs=4, space="PSUM") as psA, \
                 tc.tile_pool(name="psY", bufs=4, space="PSUM") as psY:

                def emit_so_gh_cps(gh, Hk, zsk, cps):
                    """state_out matmuls + fused (bias, z-gate) for cps"""
                    for cp in cps:
                        sl = slice(2 * cp, 2 * cp + 2)
                        ps = psA.tile([P, 2, T], F32, tag="mm", name="ps")
                        for hh in range(2):
                            nc.tensor.matmul(
                                ps[:, hh, :], blk[:, 5, :], Hk[:, 2 * cp + hh, :],
                                start=True, stop=True)
                        nc.vector.scalar_tensor_tensor(
                            gh[:, sl, :], ps[:], bias[:, 13:14],
                            zsk[:, sl, :], ALU.add, ALU.mult)

                def emit_so_gh(Hk, zsk):
                    gh = pgh.tile([P, 8, T], BF16, tag="gh", name="gh")
                    emit_so_gh_cps(gh, Hk, zsk, range(4))
                    return gh

                def emit_outproj(c, ghk, sts=(0, 1)):
                    """out_proj (gh stationary) + store chunk c"""
                    for st in sts:
                        pyA = psY.tile([P, 512], F32, tag="y", name="pyA")
                        pyB = psY.tile([P, 512], F32, tag="y", name="pyB")
                        for ct in range(8):
                            lh = ghk[:, ct, st * P:(st + 1) * P]
                            nc.tensor.matmul(pyA[:], lh, w_out[:, ct, 0:512],
                                             start=(ct == 0), stop=(ct == 7))
                            nc.tensor.matmul(pyB[:], lh, w_out[:, ct, 512:1024],
                                             start=(ct == 0), stop=(ct == 7))
                        ysb = pysb.tile([P, H], F32, tag="ysb", name="ysb")
                        if st == 0:
                            nc.scalar.activation(ysb[:, 0:512], pyA[:], AF.Copy)
                            nc.scalar.activation(ysb[:, 512:1024], pyB[:], AF.Copy)
                        else:
                            nc.vector.tensor_copy(ysb[:, 0:512], pyA[:])
                            nc.vector.tensor_copy(ysb[:, 512:1024], pyB[:])
                        r0 = c * T + st * P
                        nc.sync.dma_start(y_d[r0:r0 + P, :], ysb[:])

                xp_prev = None
                H_prev = None
                so_pend = None   # (H, zs) awaiting state_out+gh
                op_pend = None   # (c, gh) awaiting out_proj
                for c in range(NB):
                    if c == 0:
                        xT = xT_first
                    else:
                        xT = pxT.tile([P, 8, T], BF16, tag="xT", name="xT")
                        nc.sync.dma_start(xT[:], xT_d[c])

                    # x_path buffer with 3-column causal halo
                    xp = pxp.tile([P, 8, 3 + T], BF16, tag="xp", name="xp")
                    if c == 0:
                        nc.vector.memset(xp[:, :, :3], 0.0)
                    else:
                        nc.vector.tensor_copy(xp[:, :, :3], xp_prev[:, :, T:T + 3])
                    zs = pzs.tile([P, 8, T], BF16, tag="zs", name="zs")

                    # ---- in_proj x-half, then state_out+gh of the
                    # previous chunk (so the so-PSUM drains on DVE before
                    # the casts finish and bb needs the banks), then the
                    # z-half ----
                    gh_tile = (pgh.tile([P, 8, T], BF16, tag="gh", name="gh")
                               if so_pend is not None else None)
                    for jp in range(8):
                        pm = psA.tile([P, 2, T], F32, tag="mm", name="pm")
                        for hh in range(2):
                            jt = 2 * jp + hh
                            for kt in range(8):
                                nc.tensor.matmul(
                                    pm[:, hh, :],
                                    w_in_t[kt][:, jt * P:(jt + 1) * P],
                                    xT[:, kt, :],
                                    start=(kt == 0), stop=(kt == 7))
                        if jp < 4:
                            nc.vector.tensor_copy(
                                xp[:, 2 * jp:2 * jp + 2, 3:], pm[:])
                        else:
                            nc.scalar.activation(
                                zs[:, 2 * (jp - 4):2 * (jp - 4) + 2, :], pm[:],
                                AF.Silu)
                        if jp == 3 and so_pend is not None:
                            emit_so_gh_cps(gh_tile, *so_pend, range(4))

                    # ---- depthwise causal conv (bias folded into silu) ----
                    xh = pxh.tile([P, 8, T], BF16, tag="xh", name="xh")
                    for cp in range(4):
                        pc = psA.tile([P, 2, T], F32, tag="mm", name="pc")
                        for hh in range(2):
                            ct = 2 * cp + hh
                            for tap in range(K):
                                nc.tensor.matmul(
                                    pc[:, hh, :], cdiag[:, ct, tap, :],
                                    xp[:, ct, tap:tap + T],
                                    start=(tap == 0), stop=(tap == K - 1))
                        for hh in range(2):
                            ct = 2 * cp + hh
                            nc.scalar.activation(
                                xh[:, ct, :], pc[:, hh, :], AF.Silu,
                                bias=bias[:, ct:ct + 1])

                    op_next = gh_tile

                    # ---- backbone ----
                    bbt = pbb.tile([P, 8, T], BF16, tag="bbt", name="bbt")
                    for cp in range(4):
                        sl = slice(2 * cp, 2 * cp + 2)
                        pb = psA.tile([P, 2, T], F32, tag="mm", name="pb")
                        for hh in range(2):
                            nc.tensor.matmul(
                                pb[:, hh, :], blk[:, 0, :], xh[:, 2 * cp + hh, :],
                                start=True, stop=True)
                        nc.scalar.activation(
                            bbt[:, sl, :], pb[:], AF.Silu, bias=bias[:, 8:9])

                    # ---- gate matmuls + tanh ----
                    f1t = pgt.tile([P, 8, T], BF16, tag="f1", name="f1t")
                    f2t = pgt.tile([P, 8, T], BF16, tag="f2", name="f2t")
                    Ttt = pgt.tile([P, 8, T], BF16, tag="Tt", name="Ttt")
                    Tdt = pgt.tile([P, 8, T], BF16, tag="Td", name="Tdt")
                    gates = [
                        (1, f1t, 9, 1.0),
                        (2, f2t, 10, 1.0),
                        (3, Ttt, 11, 0.5),
                        (4, Tdt, 12, 0.5),
                    ]
                    # Per ct-pair: gate matmuls + tanh, then algebra + scans,
                    # so the scan chain starts right after the first pair's
                    # tanh instead of after all four.
                    # Algebra is plain tensor_tensor / tensor_scalar on DVE so
                    # the 2x_1P 16-bit perf mode can engage (stt runs 1x).
                    # cand2 = 2*cand; cpn = -4*(1-d)*cand; scan carries H=4h
                    # (state_out weights pre-scaled by 0.25).
                    r2 = palg.tile([P, 8, T], BF16, tag="r2", name="r2")
                    s2 = palg.tile([P, 8, T], BF16, tag="s2", name="s2")
                    q = palg.tile([P, 8, T], BF16, tag="q", name="q")
                    cand2 = palg.tile([P, 8, T], BF16, tag="cand2", name="cand2")
                    ddt = palg.tile([P, 8, T], BF16, tag="ddt", name="ddt")
                    Tm1 = palg.tile([P, 8, T], BF16, tag="Tm1", name="Tm1")
                    cpn = palg.tile([P, 8, T], BF16, tag="cpn", name="cpn")
                    Ht = pH.tile([P, 8, T], BF16, tag="H", name="Ht")
                    for cp in range(4):
                        sl = slice(2 * cp, 2 * cp + 2)
                        for wi, outt, bcol, scale in gates:
                            pg = psA.tile([P, 2, T], F32, tag="mm", name="pg")
                            for hh in range(2):
                                nc.tensor.matmul(
                                    pg[:, hh, :], blk[:, wi, :],
                                    bbt[:, 2 * cp + hh, :],
                                    start=True, stop=True)
                            nc.scalar.activation(
                                outt[:, sl, :], pg[:], AF.Tanh,
                                bias=bias[:, bcol:bcol + 1], scale=scale)
                        nc.vector.tensor_tensor(
                            r2[:, sl, :], f2t[:, sl, :], f1t[:, sl, :],
                            ALU.subtract)
                        nc.vector.tensor_tensor(
                            s2[:, sl, :], f2t[:, sl, :], f1t[:, sl, :], ALU.add)
                        nc.vector.tensor_tensor(
                            q[:, sl, :], Ttt[:, sl, :], r2[:, sl, :], ALU.mult)
                        nc.vector.tensor_tensor(
                            cand2[:, sl, :], s2[:, sl, :], q[:, sl, :], ALU.add)
                        nc.vector.tensor_scalar(
                            ddt[:, sl, :], Tdt[:, sl, :], 0.5, 0.5,
                            ALU.mult, ALU.add)
                        nc.vector.tensor_scalar_sub(
                            Tm1[:, sl, :], Tdt[:, sl, :], 1.0)
                        nc.vector.tensor_tensor(
                            cpn[:, sl, :], Tm1[:, sl, :], cand2[:, sl, :],
                            ALU.mult)
                        # scan: H = dd*H - cpn  (H = 4*h)
                        for ct in (2 * cp, 2 * cp + 1):
                            init = 0.0 if c == 0 else H_prev[:, ct, T - 1:T]
                            nc.vector.tensor_tensor_scan(
                                Ht[:, ct, :], ddt[:, ct, :], cpn[:, ct, :], init,
                                ALU.mult, ALU.subtract)

                    # ---- out_proj of previous chunk ----
                    if op_pend is not None:
                        emit_outproj(*op_pend)

                    xp_prev = xp
                    H_prev = Ht
                    so_pend = (Ht, zs)
                    op_pend = (c - 1, op_next) if op_next is not None else None

                # tail: interleave state_out/gh of the last chunk with
                # out_proj(NB-2) so the PE hides the scan+gh latency
                gh_last = pgh.tile([P, 8, T], BF16, tag="gh", name="gh")
                emit_so_gh_cps(gh_last, *so_pend, (0, 1))
                if op_pend is not None:
                    emit_outproj(op_pend[0], op_pend[1], sts=(0,))
                emit_so_gh_cps(gh_last, *so_pend, (2, 3))
                if op_pend is not None:
                    emit_outproj(op_pend[0], op_pend[1], sts=(1,))
                emit_outproj(NB - 1, gh_last)

    nc.compile()
    return nc


def _prep_shared(inputs):
    """Host-side preprocessing of the shared (weight) tensors."""
    import ml_dtypes
    f32 = np.float32
    bf16 = ml_dtypes.bfloat16

    in_proj_w = np.asarray(inputs["in_proj_w"], f32)
    conv_w = np.asarray(inputs["conv_w"], f32)
    conv_b = np.asarray(inputs["conv_b"], f32)

    w_in = in_proj_w.reshape(8, P, 2 * H).transpose(1, 0, 2)

    cdiag = np.zeros((8, K, P, P), f32)
    rng = np.arange(P)
    for ct in range(8):
        for tap in range(K):
            cdiag[ct, tap, rng, rng] = conv_w[ct * P:(ct + 1) * P, 0, tap]
    cdiag = cdiag.transpose(2, 0, 1, 3)  # (P, 8, K, P)

    w_out = np.asarray(inputs["out_proj_w"], f32).reshape(8, P, H).transpose(1, 0, 2)

    def blk2(w):
        o = np.zeros((P, P), f32)
        o[:64, :64] = w
        o[64:, 64:] = w
        return o

    blk = np.stack([
        blk2(np.asarray(inputs["bb_w"], f32)),
        blk2(np.asarray(inputs["f1_w"], f32)),
        blk2(np.asarray(inputs["f2_w"], f32)),
        blk2(np.asarray(inputs["tau_a_w"], f32)),
        blk2(np.asarray(inputs["decay_w"], f32)),
        blk2(np.asarray(inputs["state_out_w"], f32) * 0.25),  # scan carries 4h
    ], axis=1)  # (P, 6, P)

    def t2(v):
        return np.tile(np.asarray(v, f32), 2)

    bias = np.zeros((P, 14), f32)
    bias[:, 0:8] = conv_b.reshape(8, P).T
    bias[:, 8] = t2(inputs["bb_b"])
    bias[:, 9] = t2(inputs["f1_b"])
    bias[:, 10] = t2(inputs["f2_b"])
    bias[:, 11] = 0.5 * (t2(inputs["tau_a_b"]) + t2(inputs["tau_b"]))
    bias[:, 12] = 0.5 * t2(inputs["decay_b"])
    bias[:, 13] = t2(inputs["state_out_b"])

    return {
        "w_in": np.ascontiguousarray(w_in.astype(bf16)),
        "cdiag": np.ascontiguousarray(cdiag.astype(bf16)),
        "blk": np.ascontiguousarray(blk.astype(bf16)),
        "w_out": np.ascontiguousarray(w_out.astype(bf16)),
        "bias": np.ascontiguousarray(bias),
    }




def _make_in_maps(inputs):
    import ml_dtypes

    shared = _prep_shared(inputs)
    x = np.asarray(inputs["x"], np.float32)
    in_maps = []
    for b in range(N_CORES):
        m = dict(shared)
        xT = x[b].T.reshape(8, P, S).transpose(1, 0, 2)  # (P, 8, S) feature-major
        xTc = xT.reshape(P, 8, NB, S // NB).transpose(2, 0, 1, 3)
        m["xT"] = np.ascontiguousarray(xTc.astype(ml_dtypes.bfloat16))
        in_maps.append(m)
    return in_maps

def kernel(**inputs) -> np.ndarray:
    from concourse import bass_utils

    if "nc" not in _CACHE:
        _CACHE["nc"] = _build_program()
    nc = _CACHE["nc"]

    in_maps = _make_in_maps(inputs)
    res = bass_utils.run_bass_kernel_spmd(nc, in_maps, core_ids=list(range(N_CORES)))
    out = np.stack([res.results[b]["y"] for b in range(N_CORES)], axis=0)
    return out.astype(np.float32)



# revision 41
# speedup vs baseline: 1.1077x; 1.1077x over previous
"""CfC Liquid Cell kernel for Trainium2 (Bass/Tile), 8 NeuronCores.

Sharding: data-parallel over batch (B=8 -> 1 batch element per core).

Single fused loop over time chunks of T=256 (8 chunks), per core:
  - x is pre-transposed (and cast to bf16) on the host -> no PE transposes
  - in_proj matmuls (bf16 weights stationary, x^T moving) -> xz^T in PSUM
  - x_path half copied to SBUF (with causal halo), z half silu'd
  - depthwise causal conv = 4 shifted diagonal matmuls on PE; conv bias is
    folded into the silu ACT bias (no bias tap matmul)
  - head matmuls (bb/f1/f2/tau/decay/state_out) with 2-head block-diagonal
    64x64 weights -> full 128-partition tiles
  - sigmoid(u) = 0.5 + 0.5*tanh(u/2) via ACT scale=0.5; the scan carries
    H = 2*h (state_out weights pre-scaled by 0.5) so the gate algebra is
    only 5 fused DVE/GPSIMD passes:
       r2 = f2 - f1
       v  = (Tt + 1) * r2          # = 2*tau*(f2-f1)
       cand = 0.5*v + f1
       dd  = 0.5*Td + 0.5          # = decay
       cpn = (Td - 1) * cand       # = -2*(1-decay)*cand
       H_t = dd*H_{t-1} - cpn      # = dd*H + 2(1-dd)*cand
  - state_out bias + z-gating fused in one scalar_tensor_tensor that reads
    the state_out PSUM directly
  - out_proj with gated activations as the stationary operand -> y is
    produced time-major and DMA'd straight out
"""

import numpy as np

B, S, H = 8, 2048, 1024
NH, HD, NS, K = 16, 64, 64, 4
N_CORES = 8
T = 256             # time chunk
NB = S // T         # 8
P = 128

_CACHE = {}


def _build_program():
    import concourse.bacc as bacc
    import concourse.mybir as mybir
    import concourse.tile as tile

    F32 = mybir.dt.float32
    BF16 = mybir.dt.bfloat16
    AF = mybir.ActivationFunctionType
    ALU = mybir.AluOpType

    nc = bacc.Bacc("TRN2", target_bir_lowering=False, debug=False)

    xT_d = nc.dram_tensor("xT", (NB, P, 8, S // NB), BF16, kind="ExternalInput").ap()
    w_in_d = nc.dram_tensor("w_in", (P, 8, 2 * H), BF16, kind="ExternalInput").ap()
    cdiag_d = nc.dram_tensor("cdiag", (P, 8, K, P), BF16, kind="ExternalInput").ap()
    blk_d = nc.dram_tensor("blk", (P, 6, P), BF16, kind="ExternalInput").ap()
    w_out_d = nc.dram_tensor("w_out", (P, 8, H), BF16, kind="ExternalInput").ap()
    bias_d = nc.dram_tensor("bias", (P, 14), F32, kind="ExternalInput").ap()
    y_d = nc.dram_tensor("y", (S, H), F32, kind="ExternalOutput").ap()

    with tile.TileContext(nc) as tc:
        with tc.tile_pool(name="const", bufs=1) as cpool, \
             tc.tile_pool(name="pxT", bufs=2) as pxT0:
            # prefetch the first x chunk ahead of the big weight DMAs
            xT_first = pxT0.tile([P, 8, T], BF16, tag="xT", name="xT_first")
            nc.sync.dma_start(xT_first[:], xT_d[0])
            w_in_t = [cpool.tile([P, 2 * H], BF16, name=f"w_in{k}") for k in range(8)]
            for kt in range(8):
                eng = nc.scalar if kt % 2 == 0 else nc.sync
                eng.dma_start(w_in_t[kt][:], w_in_d[:, kt, :])
            bias = cpool.tile([P, 14], F32)
            nc.sync.dma_start(bias[:], bias_d[:])
            cdiag = cpool.tile([P, 8, K, P], BF16)
            nc.sync.dma_start(cdiag[:, 0:4], cdiag_d[:, 0:4])
            nc.scalar.dma_start(cdiag[:, 4:8], cdiag_d[:, 4:8])
            blk = cpool.tile([P, 6, P], BF16)
            nc.scalar.dma_start(blk[:], blk_d[:])
            w_out = cpool.tile([P, 8, H], BF16)
            nc.sync.dma_start(w_out[:, 0:4, :], w_out_d[:, 0:4, :])
            nc.scalar.dma_start(w_out[:, 4:8, :], w_out_d[:, 4:8, :])

            pxT = pxT0
            with \
                 tc.tile_pool(name="pxp", bufs=2) as pxp, \
                 tc.tile_pool(name="pzs", bufs=3) as pzs, \
                 tc.tile_pool(name="pxh", bufs=2) as pxh, \
                 tc.tile_pool(name="pbb", bufs=2) as pbb, \
                 tc.tile_pool(name="pgt", bufs=2) as pgt, \
                 tc.tile_pool(name="palg", bufs=1) as palg, \
                 tc.tile_pool(name="pH", bufs=2) as pH, \
                 tc.tile_pool(name="pgh", bufs=2) as pgh, \
                 tc.tile_pool(name="pysb", bufs=3) as pysb, \
                 tc.tile_pool(name="psA", bufs=4, space="PSUM") as psA, \
                 tc.tile_pool(name="psY", bufs=4, space="PSUM") as psY:

                def emit_so_gh_cps(gh, Hk, zsk, cps):
                    """state_out matmuls + fused (bias, z-gate) for cps"""
                    for cp in cps:
                        sl = slice(2 * cp, 2 * cp + 2)
                        ps = psA.tile([P, 2, T], F32, tag="mm", name="ps")
                        for hh in range(2):
                            nc.tensor.matmul(
                                ps[:, hh, :], blk[:, 5, :], Hk[:, 2 * cp + hh, :],
                                start=True, stop=True)
                        nc.vector.scalar_tensor_tensor(
                            gh[:, sl, :], ps[:], bias[:, 13:14],
                            zsk[:, sl, :], ALU.add, ALU.mult)

                def emit_so_gh(Hk, zsk):
                    gh = pgh.tile([P, 8, T], BF16, tag="gh", name="gh")
                    emit_so_gh_cps(gh, Hk, zsk, range(4))
                    return gh

                def emit_outproj(c, ghk, sts=(0, 1)):
                    """out_proj (gh stationary) + store chunk c"""
                    for st in sts:
                        pyA = psY.tile([P, 512], F32, tag="y", name="pyA")
                        pyB = psY.tile([P, 512], F32, tag="y", name="pyB")
                        for ct in range(8):
                            lh = ghk[:, ct, st * P:(st + 1) * P]
                            nc.tensor.matmul(pyA[:], lh, w_out[:, ct, 0:512],
                                             start=(ct == 0), stop=(ct == 7))
                            nc.tensor.matmul(pyB[:], lh, w_out[:, ct, 512:1024],
                                             start=(ct == 0), stop=(ct == 7))
                        ysb = pysb.tile([P, H], F32, tag="ysb", name="ysb")
                        if st == 0:
                            nc.scalar.activation(ysb[:, 0:512], pyA[:], AF.Copy)
                            nc.scalar.activation(ysb[:, 512:1024], pyB[:], AF.Copy)
                        else:
                            nc.vector.tensor_copy(ysb[:, 0:512], pyA[:])
                            nc.vector.tensor_copy(ysb[:, 512:1024], pyB[:])
                        r0 = c * T + st * P
                        nc.sync.dma_start(y_d[r0:r0 + P, :], ysb[:])

                xp_prev = None
                H_prev = None
                so_pend = None   # (H, zs) awaiting state_out+gh
                op_pend = None   # (c, gh) awaiting out_proj
                for c in range(NB):
                    if c == 0:
                        xT = xT_first
                    else:
                        xT = pxT.tile([P, 8, T], BF16, tag="xT", name="xT")
                        nc.sync.dma_start(xT[:], xT_d[c])

                    # x_path buffer with 3-column causal halo
                    xp = pxp.tile([P, 8, 3 + T], BF16, tag="xp", name="xp")
                    if c == 0:
                        nc.vector.memset(xp[:, :, :3], 0.0)
                    else:
                        nc.vector.tensor_copy(xp[:, :, :3], xp_prev[:, :, T:T + 3])
                    zs = pzs.tile([P, 8, T], BF16, tag="zs", name="zs")

                    # ---- in_proj ----
                    for jp in range(8):
                        pm = psA.tile([P, 2, T], F32, tag="mm", name="pm")
                        for hh in range(2):
                            jt = 2 * jp + hh
                            for kt in range(8):
                                nc.tensor.matmul(
                                    pm[:, hh, :],
                                    w_in_t[kt][:, jt * P:(jt + 1) * P],
                                    xT[:, kt, :],
                                    start=(kt == 0), stop=(kt == 7))
                        if jp < 4:
                            nc.vector.tensor_copy(
                                xp[:, 2 * jp:2 * jp + 2, 3:], pm[:])
                        else:
                            nc.scalar.activation(
                                zs[:, 2 * (jp - 4):2 * (jp - 4) + 2, :], pm[:],
                                AF.Silu)

                    # ---- depthwise causal conv (bias folded into silu) ----
                    xh = pxh.tile([P, 8, T], BF16, tag="xh", name="xh")
                    for cp in range(4):
                        pc = psA.tile([P, 2, T], F32, tag="mm", name="pc")
                        for hh in range(2):
                            ct = 2 * cp + hh
                            for tap in range(K):
                                nc.tensor.matmul(
                                    pc[:, hh, :], cdiag[:, ct, tap, :],
                                    xp[:, ct, tap:tap + T],
                                    start=(tap == 0), stop=(tap == K - 1))
                        for hh in range(2):
                            ct = 2 * cp + hh
                            nc.scalar.activation(
                                xh[:, ct, :], pc[:, hh, :], AF.Silu,
                                bias=bias[:, ct:ct + 1])

                    # ---- state_out + gh of previous chunk ----
                    if so_pend is not None:
                        op_next = emit_so_gh(*so_pend)
                    else:
                        op_next = None

                    # ---- backbone ----
                    bbt = pbb.tile([P, 8, T], BF16, tag="bbt", name="bbt")
                    for cp in range(4):
                        sl = slice(2 * cp, 2 * cp + 2)
                        pb = psA.tile([P, 2, T], F32, tag="mm", name="pb")
                        for hh in range(2):
                            nc.tensor.matmul(
                                pb[:, hh, :], blk[:, 0, :], xh[:, 2 * cp + hh, :],
                                start=True, stop=True)
                        nc.scalar.activation(
                            bbt[:, sl, :], pb[:], AF.Silu, bias=bias[:, 8:9])

                    # ---- gate matmuls + tanh ----
                    f1t = pgt.tile([P, 8, T], BF16, tag="f1", name="f1t")
                    f2t = pgt.tile([P, 8, T], BF16, tag="f2", name="f2t")
                    Ttt = pgt.tile([P, 8, T], BF16, tag="Tt", name="Ttt")
                    Tdt = pgt.tile([P, 8, T], BF16, tag="Td", name="Tdt")
                    gates = [
                        (1, f1t, 9, 1.0),
                        (2, f2t, 10, 1.0),
                        (3, Ttt, 11, 0.5),
                        (4, Tdt, 12, 0.5),
                    ]
                    # Per ct-pair: gate matmuls + tanh, then algebra + scans,
                    # so the scan chain starts right after the first pair's
                    # tanh instead of after all four.
                    # Algebra is plain tensor_tensor / tensor_scalar on DVE so
                    # the 2x_1P 16-bit perf mode can engage (stt runs 1x).
                    # cand2 = 2*cand; cpn = -4*(1-d)*cand; scan carries H=4h
                    # (state_out weights pre-scaled by 0.25).
                    r2 = palg.tile([P, 8, T], BF16, tag="r2", name="r2")
                    s2 = palg.tile([P, 8, T], BF16, tag="s2", name="s2")
                    q = palg.tile([P, 8, T], BF16, tag="q", name="q")
                    cand2 = palg.tile([P, 8, T], BF16, tag="cand2", name="cand2")
                    ddt = palg.tile([P, 8, T], BF16, tag="ddt", name="ddt")
                    Tm1 = palg.tile([P, 8, T], BF16, tag="Tm1", name="Tm1")
                    cpn = palg.tile([P, 8, T], BF16, tag="cpn", name="cpn")
                    Ht = pH.tile([P, 8, T], BF16, tag="H", name="Ht")
                    for cp in range(4):
                        sl = slice(2 * cp, 2 * cp + 2)
                        for wi, outt, bcol, scale in gates:
                            pg = psA.tile([P, 2, T], F32, tag="mm", name="pg")
                            for hh in range(2):
                                nc.tensor.matmul(
                                    pg[:, hh, :], blk[:, wi, :],
                                    bbt[:, 2 * cp + hh, :],
                                    start=True, stop=True)
                            nc.scalar.activation(
                                outt[:, sl, :], pg[:], AF.Tanh,
                                bias=bias[:, bcol:bcol + 1], scale=scale)
                        nc.vector.tensor_tensor(
                            r2[:, sl, :], f2t[:, sl, :], f1t[:, sl, :],
                            ALU.subtract)
                        nc.vector.tensor_tensor(
                            s2[:, sl, :], f2t[:, sl, :], f1t[:, sl, :], ALU.add)
                        nc.vector.tensor_tensor(
                            q[:, sl, :], Ttt[:, sl, :], r2[:, sl, :], ALU.mult)
                        nc.vector.tensor_tensor(
                            cand2[:, sl, :], s2[:, sl, :], q[:, sl, :], ALU.add)
                        nc.vector.tensor_scalar(
                            ddt[:, sl, :], Tdt[:, sl, :], 0.5, 0.5,
                            ALU.mult, ALU.add)
                        nc.vector.tensor_scalar_sub(
                            Tm1[:, sl, :], Tdt[:, sl, :], 1.0)
                        nc.vector.tensor_tensor(
                            cpn[:, sl, :], Tm1[:, sl, :], cand2[:, sl, :],
                            ALU.mult)
                        # scan: H = dd*H - cpn  (H = 4*h)
                        for ct in (2 * cp, 2 * cp + 1):
                            init = 0.0 if c == 0 else H_prev[:, ct, T - 1:T]
                            nc.vector.tensor_tensor_scan(
                                Ht[:, ct, :], ddt[:, ct, :], cpn[:, ct, :], init,
                                ALU.mult, ALU.subtract)

                    # ---- out_proj of previous chunk ----
                    if op_pend is not None:
                        emit_outproj(*op_pend)

                    xp_prev = xp
                    H_prev = Ht
                    so_pend = (Ht, zs)
                    op_pend = (c - 1, op_next) if op_next is not None else None

                # tail: out_proj(NB-2) pieces first so the PE covers the
                # DVE algebra/scan backlog of the last chunk, with the
                # state_out/gh pieces of the last chunk woven between
                gh_last = pgh.tile([P, 8, T], BF16, tag="gh", name="gh")
                if op_pend is not None:
                    emit_outproj(op_pend[0], op_pend[1], sts=(0,))
                emit_so_gh_cps(gh_last, *so_pend, (0, 1))
                if op_pend is not None:
                    emit_outproj(op_pend[0], op_pend[1], sts=(1,))
                emit_so_gh_cps(gh_last, *so_pend, (2, 3))
                emit_outproj(NB - 1, gh_last)

    nc.compile()
    return nc


def _prep_shared(inputs):
    """Host-side preprocessing of the shared (weight) tensors."""
    import ml_dtypes
    f32 = np.float32
    bf16 = ml_dtypes.bfloat16

    in_proj_w = np.asarray(inputs["in_proj_w"], f32)
    conv_w = np.asarray(inputs["conv_w"], f32)
    conv_b = np.asarray(inputs["conv_b"], f32)

    w_in = in_proj_w.reshape(8, P, 2 * H).transpose(1, 0, 2)

    cdiag = np.zeros((8, K, P, P), f32)
    rng = np.arange(P)
    for ct in range(8):
        for tap in range(K):
            cdiag[ct, tap, rng, rng] = conv_w[ct * P:(ct + 1) * P, 0, tap]
    cdiag = cdiag.transpose(2, 0, 1, 3)  # (P, 8, K, P)

    w_out = np.asarray(inputs["out_proj_w"], f32).reshape(8, P, H).transpose(1, 0, 2)

    def blk2(w):
        o = np.zeros((P, P), f32)
        o[:64, :64] = w
        o[64:, 64:] = w
        return o

    blk = np.stack([
        blk2(np.asarray(inputs["bb_w"], f32)),
        blk2(np.asarray(inputs["f1_w"], f32)),
        blk2(np.asarray(inputs["f2_w"], f32)),
        blk2(np.asarray(inputs["tau_a_w"], f32)),
        blk2(np.asarray(inputs["decay_w"], f32)),
        blk2(np.asarray(inputs["state_out_w"], f32) * 0.25),  # scan carries 4h
    ], axis=1)  # (P, 6, P)

    def t2(v):
        return np.tile(np.asarray(v, f32), 2)

    bias = np.zeros((P, 14), f32)
    bias[:, 0:8] = conv_b.reshape(8, P).T
    bias[:, 8] = t2(inputs["bb_b"])
    bias[:, 9] = t2(inputs["f1_b"])
    bias[:, 10] = t2(inputs["f2_b"])
    bias[:, 11] = 0.5 * (t2(inputs["tau_a_b"]) + t2(inputs["tau_b"]))
    bias[:, 12] = 0.5 * t2(inputs["decay_b"])
    bias[:, 13] = t2(inputs["state_out_b"])

    return {
        "w_in": np.ascontiguousarray(w_in.astype(bf16)),
        "cdiag": np.ascontiguousarray(cdiag.astype(bf16)),
        "blk": np.ascontiguousarray(blk.astype(bf16)),
        "w_out": np.ascontiguousarray(w_out.astype(bf16)),
        "bias": np.ascontiguousarray(bias),
    }




def _make_in_maps(inputs):
    import ml_dtypes

    shared = _prep_shared(inputs)
    x = np.asarray(inputs["x"], np.float32)
    in_maps = []
    for b in range(N_CORES):
        m = dict(shared)
        xT = x[b].T.reshape(8, P, S).transpose(1, 0, 2)  # (P, 8, S) feature-major
        xTc = xT.reshape(P, 8, NB, S // NB).transpose(2, 0, 1, 3)
        m["xT"] = np.ascontiguousarray(xTc.astype(ml_dtypes.bfloat16))
        in_maps.append(m)
    return in_maps

def kernel(**inputs) -> np.ndarray:
    from concourse import bass_utils

    if "nc" not in _CACHE:
        _CACHE["nc"] = _build_program()
    nc = _CACHE["nc"]

    in_maps = _make_in_maps(inputs)
    res = bass_utils.run_bass_kernel_spmd(nc, in_maps, core_ids=list(range(N_CORES)))
    out = np.stack([res.results[b]["y"] for b in range(N_CORES)], axis=0)
    return out.astype(np.float32)



# revision 42
# speedup vs baseline: 1.1338x; 1.0236x over previous
"""CfC Liquid Cell kernel for Trainium2 (Bass/Tile), 8 NeuronCores.

Sharding: data-parallel over batch (B=8 -> 1 batch element per core).

Single fused loop over time chunks of T=256 (8 chunks), per core:
  - x is pre-transposed (and cast to bf16) on the host -> no PE transposes
  - in_proj matmuls (bf16 weights stationary, x^T moving) -> xz^T in PSUM
  - x_path half copied to SBUF (with causal halo), z half silu'd
  - depthwise causal conv = 4 shifted diagonal matmuls on PE; conv bias is
    folded into the silu ACT bias (no bias tap matmul)
  - head matmuls (bb/f1/f2/tau/decay/state_out) with 2-head block-diagonal
    64x64 weights -> full 128-partition tiles
  - sigmoid(u) = 0.5 + 0.5*tanh(u/2) via ACT scale=0.5; the scan carries
    H = 2*h (state_out weights pre-scaled by 0.5) so the gate algebra is
    only 5 fused DVE/GPSIMD passes:
       r2 = f2 - f1
       v  = (Tt + 1) * r2          # = 2*tau*(f2-f1)
       cand = 0.5*v + f1
       dd  = 0.5*Td + 0.5          # = decay
       cpn = (Td - 1) * cand       # = -2*(1-decay)*cand
       H_t = dd*H_{t-1} - cpn      # = dd*H + 2(1-dd)*cand
  - state_out bias + z-gating fused in one scalar_tensor_tensor that reads
    the state_out PSUM directly
  - out_proj with gated activations as the stationary operand -> y is
    produced time-major and DMA'd straight out
"""

import numpy as np

B, S, H = 8, 2048, 1024
NH, HD, NS, K = 16, 64, 64, 4
N_CORES = 8
T = 256             # time chunk
NB = S // T         # 8
P = 128

_CACHE = {}


def _build_program():
    import concourse.bacc as bacc
    import concourse.mybir as mybir
    import concourse.tile as tile

    F32 = mybir.dt.float32
    BF16 = mybir.dt.bfloat16
    AF = mybir.ActivationFunctionType
    ALU = mybir.AluOpType

    nc = bacc.Bacc("TRN2", target_bir_lowering=False, debug=False)

    xT_d = nc.dram_tensor("xT", (NB, P, 8, S // NB), BF16, kind="ExternalInput").ap()
    w_in_d = nc.dram_tensor("w_in", (P, 8, 2 * H), BF16, kind="ExternalInput").ap()
    cdiag_d = nc.dram_tensor("cdiag", (P, 8, K, P), BF16, kind="ExternalInput").ap()
    blk_d = nc.dram_tensor("blk", (P, 6, P), BF16, kind="ExternalInput").ap()
    w_out_d = nc.dram_tensor("w_out", (P, 8, H), BF16, kind="ExternalInput").ap()
    bias_d = nc.dram_tensor("bias", (P, 14), F32, kind="ExternalInput").ap()
    y_d = nc.dram_tensor("y", (S, H), F32, kind="ExternalOutput").ap()

    with tile.TileContext(nc) as tc:
        with tc.tile_pool(name="const", bufs=1) as cpool, \
             tc.tile_pool(name="pxT", bufs=2) as pxT0:
            # prefetch the first x chunk ahead of the big weight DMAs
            xT_first = pxT0.tile([P, 8, T], BF16, tag="xT", name="xT_first")
            nc.sync.dma_start(xT_first[:], xT_d[0])
            w_in_t = [cpool.tile([P, 2 * H], BF16, name=f"w_in{k}") for k in range(8)]
            for kt in range(8):
                eng = nc.scalar if kt % 2 == 0 else nc.sync
                eng.dma_start(w_in_t[kt][:], w_in_d[:, kt, :])
            bias = cpool.tile([P, 14], F32)
            nc.sync.dma_start(bias[:], bias_d[:])
            cdiag = cpool.tile([P, 8, K, P], BF16)
            nc.sync.dma_start(cdiag[:, 0:4], cdiag_d[:, 0:4])
            nc.scalar.dma_start(cdiag[:, 4:8], cdiag_d[:, 4:8])
            blk = cpool.tile([P, 6, P], BF16)
            nc.scalar.dma_start(blk[:], blk_d[:])
            w_out = cpool.tile([P, 8, H], BF16)
            nc.sync.dma_start(w_out[:, 0:4, :], w_out_d[:, 0:4, :])
            nc.scalar.dma_start(w_out[:, 4:8, :], w_out_d[:, 4:8, :])

            pxT = pxT0
            with \
                 tc.tile_pool(name="pxp", bufs=2) as pxp, \
                 tc.tile_pool(name="pzs", bufs=3) as pzs, \
                 tc.tile_pool(name="pxh", bufs=2) as pxh, \
                 tc.tile_pool(name="pbb", bufs=2) as pbb, \
                 tc.tile_pool(name="pgt", bufs=2) as pgt, \
                 tc.tile_pool(name="palg", bufs=1) as palg, \
                 tc.tile_pool(name="pH", bufs=2) as pH, \
                 tc.tile_pool(name="pgh", bufs=2) as pgh, \
                 tc.tile_pool(name="pysb", bufs=3) as pysb, \
                 tc.tile_pool(name="psA", bufs=5, space="PSUM") as psA, \
                 tc.tile_pool(name="psY", bufs=3, space="PSUM") as psY:

                def emit_so_gh(Hk, zsk):
                    """state_out matmul + fused (bias, z-gate) -> gh tile"""
                    gh = pgh.tile([P, 8, T], BF16, tag="gh", name="gh")
                    for cp in range(4):
                        sl = slice(2 * cp, 2 * cp + 2)
                        ps = psA.tile([P, 2, T], F32, tag="mm", name="ps")
                        for hh in range(2):
                            nc.tensor.matmul(
                                ps[:, hh, :], blk[:, 5, :], Hk[:, 2 * cp + hh, :],
                                start=True, stop=True)
                        nc.vector.scalar_tensor_tensor(
                            gh[:, sl, :], ps[:], bias[:, 13:14],
                            zsk[:, sl, :], ALU.add, ALU.mult)
                    return gh

                def emit_outproj(c, ghk):
                    """out_proj (gh stationary) + store chunk c"""
                    for st in range(2):
                        pyA = psY.tile([P, 512], F32, tag="y", name="pyA")
                        pyB = psY.tile([P, 512], F32, tag="y", name="pyB")
                        for ct in range(8):
                            lh = ghk[:, ct, st * P:(st + 1) * P]
                            nc.tensor.matmul(pyA[:], lh, w_out[:, ct, 0:512],
                                             start=(ct == 0), stop=(ct == 7))
                            nc.tensor.matmul(pyB[:], lh, w_out[:, ct, 512:1024],
                                             start=(ct == 0), stop=(ct == 7))
                        ysb = pysb.tile([P, H], F32, tag="ysb", name="ysb")
                        if st == 0:
                            nc.scalar.activation(ysb[:, 0:512], pyA[:], AF.Copy)
                            nc.scalar.activation(ysb[:, 512:1024], pyB[:], AF.Copy)
                        else:
                            nc.vector.tensor_copy(ysb[:, 0:512], pyA[:])
                            nc.vector.tensor_copy(ysb[:, 512:1024], pyB[:])
                        r0 = c * T + st * P
                        nc.sync.dma_start(y_d[r0:r0 + P, :], ysb[:])

                xp_prev = None
                H_prev = None
                so_pend = None   # (H, zs) awaiting state_out+gh
                op_pend = None   # (c, gh) awaiting out_proj
                for c in range(NB):
                    if c == 0:
                        xT = xT_first
                    else:
                        xT = pxT.tile([P, 8, T], BF16, tag="xT", name="xT")
                        nc.sync.dma_start(xT[:], xT_d[c])

                    # x_path buffer with 3-column causal halo
                    xp = pxp.tile([P, 8, 3 + T], BF16, tag="xp", name="xp")
                    if c == 0:
                        nc.vector.memset(xp[:, :, :3], 0.0)
                    else:
                        nc.vector.tensor_copy(xp[:, :, :3], xp_prev[:, :, T:T + 3])
                    zs = pzs.tile([P, 8, T], BF16, tag="zs", name="zs")

                    # ---- in_proj ----
                    for jp in range(8):
                        pm = psA.tile([P, 2, T], F32, tag="mm", name="pm")
                        for hh in range(2):
                            jt = 2 * jp + hh
                            for kt in range(8):
                                nc.tensor.matmul(
                                    pm[:, hh, :],
                                    w_in_t[kt][:, jt * P:(jt + 1) * P],
                                    xT[:, kt, :],
                                    start=(kt == 0), stop=(kt == 7))
                        if jp < 4:
                            nc.vector.tensor_copy(
                                xp[:, 2 * jp:2 * jp + 2, 3:], pm[:])
                        else:
                            nc.scalar.activation(
                                zs[:, 2 * (jp - 4):2 * (jp - 4) + 2, :], pm[:],
                                AF.Silu)

                    # ---- depthwise causal conv (bias folded into silu) ----
                    xh = pxh.tile([P, 8, T], BF16, tag="xh", name="xh")
                    for cp in range(4):
                        pc = psA.tile([P, 2, T], F32, tag="mm", name="pc")
                        for hh in range(2):
                            ct = 2 * cp + hh
                            for tap in range(K):
                                nc.tensor.matmul(
                                    pc[:, hh, :], cdiag[:, ct, tap, :],
                                    xp[:, ct, tap:tap + T],
                                    start=(tap == 0), stop=(tap == K - 1))
                        for hh in range(2):
                            ct = 2 * cp + hh
                            nc.scalar.activation(
                                xh[:, ct, :], pc[:, hh, :], AF.Silu,
                                bias=bias[:, ct:ct + 1])

                    # ---- state_out + gh of previous chunk (scan done long ago)
                    if so_pend is not None:
                        op_next = emit_so_gh(*so_pend)
                    else:
                        op_next = None

                    # ---- backbone ----
                    bbt = pbb.tile([P, 8, T], BF16, tag="bbt", name="bbt")
                    for cp in range(4):
                        sl = slice(2 * cp, 2 * cp + 2)
                        pb = psA.tile([P, 2, T], F32, tag="mm", name="pb")
                        for hh in range(2):
                            nc.tensor.matmul(
                                pb[:, hh, :], blk[:, 0, :], xh[:, 2 * cp + hh, :],
                                start=True, stop=True)
                        nc.scalar.activation(
                            bbt[:, sl, :], pb[:], AF.Silu, bias=bias[:, 8:9])

                    # ---- gate matmuls + tanh ----
                    f1t = pgt.tile([P, 8, T], BF16, tag="f1", name="f1t")
                    f2t = pgt.tile([P, 8, T], BF16, tag="f2", name="f2t")
                    Ttt = pgt.tile([P, 8, T], BF16, tag="Tt", name="Ttt")
                    Tdt = pgt.tile([P, 8, T], BF16, tag="Td", name="Tdt")
                    gates = [
                        (1, f1t, 9, 1.0),
                        (2, f2t, 10, 1.0),
                        (3, Ttt, 11, 0.5),
                        (4, Tdt, 12, 0.5),
                    ]
                    # Per ct-pair: gate matmuls + tanh, then algebra + scans,
                    # so the scan chain starts right after the first pair's
                    # tanh instead of after all four.
                    # Algebra is plain tensor_tensor / tensor_scalar on DVE so
                    # the 2x_1P 16-bit perf mode can engage (stt runs 1x).
                    # cand2 = 2*cand; cpn = -4*(1-d)*cand; scan carries H=4h
                    # (state_out weights pre-scaled by 0.25).
                    r2 = palg.tile([P, 8, T], BF16, tag="r2", name="r2")
                    s2 = palg.tile([P, 8, T], BF16, tag="s2", name="s2")
                    q = palg.tile([P, 8, T], BF16, tag="q", name="q")
                    cand2 = palg.tile([P, 8, T], BF16, tag="cand2", name="cand2")
                    ddt = palg.tile([P, 8, T], BF16, tag="ddt", name="ddt")
                    Tm1 = palg.tile([P, 8, T], BF16, tag="Tm1", name="Tm1")
                    cpn = palg.tile([P, 8, T], BF16, tag="cpn", name="cpn")
                    Ht = pH.tile([P, 8, T], BF16, tag="H", name="Ht")
                    for cp in range(4):
                        sl = slice(2 * cp, 2 * cp + 2)
                        for wi, outt, bcol, scale in gates:
                            pg = psA.tile([P, 2, T], F32, tag="mm", name="pg")
                            for hh in range(2):
                                nc.tensor.matmul(
                                    pg[:, hh, :], blk[:, wi, :],
                                    bbt[:, 2 * cp + hh, :],
                                    start=True, stop=True)
                            nc.scalar.activation(
                                outt[:, sl, :], pg[:], AF.Tanh,
                                bias=bias[:, bcol:bcol + 1], scale=scale)
                        nc.vector.tensor_tensor(
                            r2[:, sl, :], f2t[:, sl, :], f1t[:, sl, :],
                            ALU.subtract)
                        nc.vector.tensor_tensor(
                            s2[:, sl, :], f2t[:, sl, :], f1t[:, sl, :], ALU.add)
                        nc.vector.tensor_tensor(
                            q[:, sl, :], Ttt[:, sl, :], r2[:, sl, :], ALU.mult)
                        nc.vector.tensor_tensor(
                            cand2[:, sl, :], s2[:, sl, :], q[:, sl, :], ALU.add)
                        nc.vector.tensor_scalar(
                            ddt[:, sl, :], Tdt[:, sl, :], 0.5, 0.5,
                            ALU.mult, ALU.add)
                        nc.vector.tensor_scalar_sub(
                            Tm1[:, sl, :], Tdt[:, sl, :], 1.0)
                        nc.vector.tensor_tensor(
                            cpn[:, sl, :], Tm1[:, sl, :], cand2[:, sl, :],
                            ALU.mult)
                        # scan: H = dd*H - cpn  (H = 4*h)
                        for ct in (2 * cp, 2 * cp + 1):
                            init = 0.0 if c == 0 else H_prev[:, ct, T - 1:T]
                            nc.vector.tensor_tensor_scan(
                                Ht[:, ct, :], ddt[:, ct, :], cpn[:, ct, :], init,
                                ALU.mult, ALU.subtract)

                    # ---- out_proj of previous chunk ----
                    if op_pend is not None:
                        emit_outproj(*op_pend)

                    xp_prev = xp
                    H_prev = Ht
                    so_pend = (Ht, zs)
                    op_pend = (c - 1, op_next) if op_next is not None else None

                # tail
                gh_last = emit_so_gh(*so_pend)
                if op_pend is not None:
                    emit_outproj(*op_pend)
                emit_outproj(NB - 1, gh_last)

    nc.compile()
    return nc


def _prep_shared(inputs):
    """Host-side preprocessing of the shared (weight) tensors."""
    import ml_dtypes
    f32 = np.float32
    bf16 = ml_dtypes.bfloat16

    in_proj_w = np.asarray(inputs["in_proj_w"], f32)
    conv_w = np.asarray(inputs["conv_w"], f32)
    conv_b = np.asarray(inputs["conv_b"], f32)

    w_in = in_proj_w.reshape(8, P, 2 * H).transpose(1, 0, 2)

    cdiag = np.zeros((8, K, P, P), f32)
    rng = np.arange(P)
    for ct in range(8):
        for tap in range(K):
            cdiag[ct, tap, rng, rng] = conv_w[ct * P:(ct + 1) * P, 0, tap]
    cdiag = cdiag.transpose(2, 0, 1, 3)  # (P, 8, K, P)

    w_out = np.asarray(inputs["out_proj_w"], f32).reshape(8, P, H).transpose(1, 0, 2)

    def blk2(w):
        o = np.zeros((P, P), f32)
        o[:64, :64] = w
        o[64:, 64:] = w
        return o

    blk = np.stack([
        blk2(np.asarray(inputs["bb_w"], f32)),
        blk2(np.asarray(inputs["f1_w"], f32)),
        blk2(np.asarray(inputs["f2_w"], f32)),
        blk2(np.asarray(inputs["tau_a_w"], f32)),
        blk2(np.asarray(inputs["decay_w"], f32)),
        blk2(np.asarray(inputs["state_out_w"], f32) * 0.25),  # scan carries 4h
    ], axis=1)  # (P, 6, P)

    def t2(v):
        return np.tile(np.asarray(v, f32), 2)

    bias = np.zeros((P, 14), f32)
    bias[:, 0:8] = conv_b.reshape(8, P).T
    bias[:, 8] = t2(inputs["bb_b"])
    bias[:, 9] = t2(inputs["f1_b"])
    bias[:, 10] = t2(inputs["f2_b"])
    bias[:, 11] = 0.5 * (t2(inputs["tau_a_b"]) + t2(inputs["tau_b"]))
    bias[:, 12] = 0.5 * t2(inputs["decay_b"])
    bias[:, 13] = t2(inputs["state_out_b"])

    return {
        "w_in": np.ascontiguousarray(w_in.astype(bf16)),
        "cdiag": np.ascontiguousarray(cdiag.astype(bf16)),
        "blk": np.ascontiguousarray(blk.astype(bf16)),
        "w_out": np.ascontiguousarray(w_out.astype(bf16)),
        "bias": np.ascontiguousarray(bias),
    }




def _make_in_maps(inputs):
    import ml_dtypes

    shared = _prep_shared(inputs)
    x = np.asarray(inputs["x"], np.float32)
    in_maps = []
    for b in range(N_CORES):
        m = dict(shared)
        xT = x[b].T.reshape(8, P, S).transpose(1, 0, 2)  # (P, 8, S) feature-major
        xTc = xT.reshape(P, 8, NB, S // NB).transpose(2, 0, 1, 3)
        m["xT"] = np.ascontiguousarray(xTc.astype(ml_dtypes.bfloat16))
        in_maps.append(m)
    return in_maps

def kernel(**inputs) -> np.ndarray:
    from concourse import bass_utils

    if "nc" not in _CACHE:
        _CACHE["nc"] = _build_program()
    nc = _CACHE["nc"]

    in_maps = _make_in_maps(inputs)
    res = bass_utils.run_bass_kernel_spmd(nc, in_maps, core_ids=list(range(N_CORES)))
    out = np.stack([res.results[b]["y"] for b in range(N_CORES)], axis=0)
    return out.astype(np.float32)

